# revision 37
# baseline (speedup 1.0000x reference)
"""Trainium2 Bass kernel for the per-feature MLP ensemble (dense_mlp).

Reference computation (per feature f of F=128 independent tiny MLPs):
    h1 = elu(X @ W1[f] + b1[f])        X:[N,160]  W1[f]:[160,32]
    h2 = elu(h1 @ W2[f] + b2[f])       W2[f]:[32,32]
    out[:, f] = h2 @ W3[f] + b3[f]     W3[f]:[32]

Strategy:
  - Data-parallel: shard N=32768 rows across 8 cores (4096 each),
    replicate the (tiny) weights.
  - Transposed layout on chip: channels (f,h) on SBUF partitions, n on
    the free dimension.  The F networks are processed in 32 groups of 4
    features = 128 channels, so layer 2 is a single 128x128 block-diagonal
    matmul per group and layer 3 a 128->4 matmul per group.
  - ELU via the exact identity   elu(y) + 1 = max(y + 1, min(e^y, 1)).
    Layer 1 keeps psum = y + 1 (bias+1 folded into the ones row of
    xt_b), so its ELU is one ScalarE Exp (bias -1) + one VectorE
    scalar_tensor_tensor.  Layer 2 keeps psum = y - c2 (c2 = b2 -
    colsum(W2)); its ELU is Exp (per-channel bias c2) + a 2x/4x-mode
    tensor_scalar  v2 = min(e,1) - (c2+1)  + stt  u2 = max(v2, psum),
    which equals (h2+1) - (c2+1); the shift folds into b3pp = b3+W3.c2.
  - Chunk-pair iterations: each iteration processes one group x 1024
    rows (two 512 chunks).  Every weight load feeds two back-to-back
    matmuls (adjacent chunks), psum tiles are [128,1024] spanning two
    banks, and ACT/DVE ops run at FD=1024 to amortize per-op overhead.
  - A PE warm-up burst of dependency-free matmuls at kernel start gives
    the HAM clock-gate its >3.4us unbroken-busy window (1.2 -> 2.4 GHz)
    while the input DMAs are still streaming.
  - Software pipelining: stages L1 / L2 / L3 at offsets 0 / -2 / -4 so
    the PE queue always has independent matmuls ahead of the ones that
    wait on ACT/DVE results.
"""

import numpy as np

import concourse.bass as bass
import concourse.bacc as bacc
import concourse.mybir as mybir
import concourse.tile as tile
from concourse.bass_utils import run_bass_kernel_spmd

N, D, F, H = 32768, 160, 128, 32
NCORES = 8
NS = N // NCORES          # rows per core
CH = F * H                # 4096 channels after layer 1
GROUPS = F // 4           # 32 groups of 4 features (=128 channels)
CHUNK = 512               # free-dim (n) sub-tile size (one psum bank)
PAIR = 2 * CHUNK          # rows per iteration
NPAIRS = NS // PAIR
T = NPAIRS * GROUPS       # flattened (pair, group) iteration count

FP16 = mybir.dt.float16
F32 = mybir.dt.float32
AF = mybir.ActivationFunctionType
ALU = mybir.AluOpType


def _build_bass():
    nc = bacc.Bacc("TRN2", target_bir_lowering=False, debug=False,
                   num_devices=NCORES)

    def inp(name, shape, dt):
        return nc.dram_tensor(name, shape, dt, kind="ExternalInput").ap()

    xt_a = inp("xt_a", [128, NS], FP16)        # X.T rows 0..127 (shard)
    xt_b = inp("xt_b", [33, NS], FP16)         # X.T rows 128..159 + ones row
    w1a = inp("w1a", [128, CH], FP16)          # W1' rows 0..127
    w1b = inp("w1b", [33, CH], FP16)           # W1' rows 128..159 + (b1+1) row
    w2b = inp("w2b", [128, GROUPS * 128], FP16)  # blockdiag(W2) per group
    c2 = inp("c2", [128, GROUPS], F32)         # b2 - colsum(W2), per channel
    c2p1 = inp("c2p1", [128, GROUPS], F32)     # c2 + 1
    w3s = inp("w3s", [128, GROUPS * 128], FP16)  # W3 cols at out partition
    b3pp = inp("b3pp", [128, 1], F32)          # b3 + W3 . c2
    neg1 = inp("neg1", [128, 1], F32)          # constant -1 bias column
    outT = nc.dram_tensor("outT", [128, NS], F32, kind="ExternalOutput").ap()

    from contextlib import ExitStack
    with tile.TileContext(nc) as tc, ExitStack() as ctx:
        wp = ctx.enter_context(tc.tile_pool(name="w", bufs=1))

        def load(ap_dram, shape, dt, tag):
            t = wp.tile(list(shape), dt, tag=tag)
            nc.sync.dma_start(t[:], ap_dram)
            return t

        w1a_sb = load(w1a, [128, CH], FP16, "w1a")
        w1b_sb = load(w1b, [33, CH], FP16, "w1b")
        # xt_a arrives in pair-sized slices so the first L1 matmul only
        # waits for slice 0 (subtile deps cover the rest).
        xt_a_sb = wp.tile([128, NS], FP16, tag="xt_a")
        for q in range(NPAIRS):
            qs = slice(PAIR * q, PAIR * (q + 1))
            nc.sync.dma_start(xt_a_sb[:, qs], xt_a[:, qs])
        xt_b_sb = load(xt_b, [33, NS], FP16, "xt_b")
        w2b_sb = load(w2b, [128, GROUPS * 128], FP16, "w2b")
        c2_sb = load(c2, [128, GROUPS], F32, "c2")
        c2p1_sb = load(c2p1, [128, GROUPS], F32, "c2p1")
        w3s_sb = load(w3s, [128, GROUPS * 128], FP16, "w3s")
        b3_sb = load(b3pp, [128, 1], F32, "b3pp")
        neg1_sb = load(neg1, [128, 1], F32, "neg1")

        # Warm the ACT Exp table on a tiny tile so the table-load pseudo-op
        # lands on a low-dependency instruction instead of the first real
        # activation.
        warm = wp.tile([128, 1], FP16, tag="warm")
        nc.scalar.activation(warm[:], neg1_sb[:], AF.Exp,
                             bias=neg1_sb[:, 0:1])

        ip = ctx.enter_context(tc.tile_pool(name="interm", bufs=5))
        p1 = ctx.enter_context(tc.tile_pool(name="p1", bufs=2, space="PSUM"))
        p2 = ctx.enter_context(tc.tile_pool(name="p2", bufs=1, space="PSUM"))
        po = ctx.enter_context(tc.tile_pool(name="po", bufs=1, space="PSUM"))
        op = ctx.enter_context(tc.tile_pool(name="osb", bufs=2))

        # PE HAM warm-up: ~20 dependency-free back-to-back matmuls on
        # memset tiles give the clock-gate the >3.4us unbroken-busy window
        # it needs to lift the PE from 1.2 to 2.4 GHz.  They run while the
        # input DMAs are still in flight and write garbage into the psum
        # bank that the first real layer-3 accumulation later clears with
        # its start=True.
        dw = wp.tile([128, 128], FP16, tag="dw")
        nc.vector.memset(dw[:], 0.0)
        dx = wp.tile([128, CHUNK], FP16, tag="dx")
        nc.vector.memset(dx[:], 0.0)
        pwarm = po.tile([128, PAIR], F32, name="pwarm", tag="pout")
        for _ in range(20):
            nc.tensor.matmul(pwarm[:, 0:CHUNK], dw[:], dx[:],
                             start=True, stop=True)

        u1_tiles = {}
        u2_tiles = {}
        pout_tiles = {}

        def halves(t):
            P = t // GROUPS
            a = slice(PAIR * P, PAIR * P + CHUNK)
            b = slice(PAIR * P + CHUNK, PAIR * (P + 1))
            return a, b

        def g128_of(t):
            g = t % GROUPS
            return slice(128 * g, 128 * (g + 1))

        for t in range(T + 4):
            # ---------------- L1 stage (iter t) ----------------
            if t < T:
                ca, cb = halves(t)
                g128 = g128_of(t)
                ps1 = p1.tile([128, PAIR], F32)
                nc.tensor.matmul(ps1[:, 0:CHUNK], w1a_sb[:, g128],
                                 xt_a_sb[:, ca], start=True, stop=False)
                nc.tensor.matmul(ps1[:, CHUNK:PAIR], w1a_sb[:, g128],
                                 xt_a_sb[:, cb], start=True, stop=False)
                nc.tensor.matmul(ps1[:, 0:CHUNK], w1b_sb[:, g128],
                                 xt_b_sb[:, ca], start=False, stop=True)
                nc.tensor.matmul(ps1[:, CHUNK:PAIR], w1b_sb[:, g128],
                                 xt_b_sb[:, cb], start=False, stop=True)
                e1 = ip.tile([128, PAIR], FP16, tag="e1")
                nc.scalar.activation(e1[:], ps1[:], AF.Exp,
                                     bias=neg1_sb[:, 0:1])
                u1 = ip.tile([128, PAIR], FP16, tag="u1")
                nc.vector.scalar_tensor_tensor(
                    u1[:], e1[:], 1.0, ps1[:], ALU.min, ALU.max)
                u1_tiles[t] = u1
            # ---------------- L2 stage (iter t-2) ----------------
            s = t - 2
            if 0 <= s < T:
                g = s % GROUPS
                g128 = g128_of(s)
                u1 = u1_tiles.pop(s)
                ps2 = p2.tile([128, PAIR], F32)
                nc.tensor.matmul(ps2[:, 0:CHUNK], w2b_sb[:, g128],
                                 u1[:, 0:CHUNK], start=True, stop=True)
                nc.tensor.matmul(ps2[:, CHUNK:PAIR], w2b_sb[:, g128],
                                 u1[:, CHUNK:PAIR], start=True, stop=True)
                e2 = ip.tile([128, PAIR], FP16, tag="e2")
                nc.scalar.activation(e2[:], ps2[:], AF.Exp,
                                     bias=c2_sb[:, g:g + 1])
                v2 = ip.tile([128, PAIR], FP16, tag="v2")
                nc.vector.tensor_scalar(v2[:], e2[:], 1.0,
                                        c2p1_sb[:, g:g + 1],
                                        ALU.min, ALU.subtract)
                u2 = ip.tile([128, PAIR], FP16, tag="u2")
                nc.vector.scalar_tensor_tensor(
                    u2[:], v2[:], 0.0, ps2[:], ALU.add, ALU.max)
                u2_tiles[s] = u2
            # ---------------- L3 stage (iter t-4) ----------------
            s = t - 4
            if 0 <= s < T:
                P, g = s // GROUPS, s % GROUPS
                u2 = u2_tiles.pop(s)
                if g == 0:
                    pout_tiles[P] = po.tile([128, PAIR], F32,
                                            name="pout", tag="pout")
                pout = pout_tiles[P]
                w3g = w3s_sb[:, 128 * g:128 * (g + 1)]
                first = (g == 0)
                last = (g == GROUPS - 1)
                nc.tensor.matmul(pout[:, 0:CHUNK], w3g,
                                 u2[:, 0:CHUNK], start=first, stop=last)
                nc.tensor.matmul(pout[:, CHUNK:PAIR], w3g,
                                 u2[:, CHUNK:PAIR], start=first, stop=last)
                if g == GROUPS - 1:
                    osb = op.tile([128, PAIR], F32, name="osb", tag="osb")
                    nc.scalar.activation(osb[:], pout[:], AF.Identity,
                                         bias=b3_sb[:, 0:1])
                    nc.sync.dma_start(outT[:, PAIR * P:PAIR * (P + 1)],
                                      osb[:])
                    del pout_tiles[P]
    nc.compile()
    return nc


def _prep_inputs(X, W1, b1, W2, b2, W3, b3):
    X = np.asarray(X, np.float32)
    W1 = np.asarray(W1, np.float32)
    b1 = np.asarray(b1, np.float32)
    W2 = np.asarray(W2, np.float32)
    b2 = np.asarray(b2, np.float32)
    W3 = np.asarray(W3, np.float32)
    b3 = np.asarray(b3, np.float32)

    W1p = W1.transpose(1, 0, 2).reshape(D, CH)
    b1p = b1.reshape(CH)
    w1a = np.ascontiguousarray(W1p[0:128]).astype(np.float16)
    w1b = np.concatenate([W1p[128:160], (b1p + 1.0)[None, :]], 0).astype(np.float16)

    XT = X.T
    xt_a_full = np.ascontiguousarray(XT[0:128]).astype(np.float16)
    xt_b_full = np.concatenate(
        [XT[128:160], np.ones((1, N), np.float32)], 0).astype(np.float16)

    w2blk = np.zeros((128, GROUPS * 128), np.float32)
    for g in range(GROUPS):
        for j in range(4):
            f = 4 * g + j
            w2blk[32 * j:32 * (j + 1),
                  128 * g + 32 * j:128 * g + 32 * (j + 1)] = W2[f]
    w2blk = w2blk.astype(np.float16)

    colsum2 = W2.sum(axis=1)                       # [F, H]
    c2_ch = (b2 - colsum2).reshape(CH)             # per-channel c2
    c2 = np.ascontiguousarray(c2_ch.reshape(GROUPS, 128).T).astype(np.float32)
    c2p1 = (c2 + 1.0).astype(np.float32)

    w3s = np.zeros((128, GROUPS * 128), np.float32)
    for g in range(GROUPS):
        for j in range(4):
            f = 4 * g + j
            w3s[32 * j:32 * (j + 1), 128 * g + f] = W3[f]
    w3s = w3s.astype(np.float16)

    # layer-3 rhs is u2 = h2 - c2, so fold +W3.c2 into b3
    b3pp = (b3 + (W3 * c2_ch.reshape(F, H)).sum(1)).astype(np.float32)
    b3pp = b3pp.reshape(128, 1)
    neg1 = np.full((128, 1), -1.0, np.float32)

    shared = dict(w1a=w1a, w1b=w1b, w2b=w2blk, c2=c2, c2p1=c2p1, w3s=w3s,
                  b3pp=b3pp, neg1=neg1)
    in_maps = []
    for c in range(NCORES):
        sl = slice(c * NS, (c + 1) * NS)
        m = dict(shared)
        m["xt_a"] = np.ascontiguousarray(xt_a_full[:, sl])
        m["xt_b"] = np.ascontiguousarray(xt_b_full[:, sl])
        in_maps.append(m)
    return in_maps


_NC_CACHE = {}


def _get_nc():
    if "nc" not in _NC_CACHE:
        _NC_CACHE["nc"] = _build_bass()
    return _NC_CACHE["nc"]


def kernel(X, W1, b1, W2, b2, W3, b3, trace=False, trace_kwargs=None):
    nc = _get_nc()
    in_maps = _prep_inputs(X, W1, b1, W2, b2, W3, b3)
    res = run_bass_kernel_spmd(nc, in_maps, list(range(NCORES)),
                               trace=trace, **(trace_kwargs or {}))
    outs = res.results
    outT = np.concatenate([outs[c]["outT"] for c in range(NCORES)], axis=1)
    out = np.ascontiguousarray(outT.T).astype(np.float32)
    if trace:
        kernel.last_results = res
    return out
